# revision 1
# baseline (speedup 1.0000x reference)
"""Trainium2 Bass kernel for a 2-layer GCN (GCNConv x2 + MLP head),
8-core SPMD via run_bass_kernel_spmd.

Entry point: kernel(**inputs) -> np.ndarray [100000, 4] float32.

Sharding: nodes are partitioned across the 8 cores (12500 each). Each core
projects its slice (x @ W1), the slices are AllGathered on-device, and each
core aggregates messages for its destination range by indirect-DMA gathering
source rows and accumulating them into [feat, dst] PSUM blocks with a
selection-matrix matmul (S[e, d] = (iota == dstloc_e) * norm_e built in one
fused DVE tensor_scalar op). Layer 2 repeats the pattern; the MLP head is
fused per destination block. All aggregation outputs live transposed, which
is exactly the lhsT layout the following matmul needs, so the kernel contains
no transposes. Graph preprocessing (self-loops, degree normalization,
destination-sorted chunk packing) happens on the host inside kernel().
"""

import numpy as np

import concourse.bass as bass
import concourse.bacc as bacc
import concourse.mybir as mybir
import concourse.tile as tile

F32 = mybir.dt.float32
I32 = mybir.dt.int32


def default_cfg():
    return dict(N=100000, NCORES=8, IN_DIM=518, HID=128, MID=64, OUT=4)


def derive(cfg):
    cfg = dict(cfg)
    cfg["NPC"] = cfg["N"] // cfg["NCORES"]
    cfg["NB"] = (cfg["NPC"] + 127) // 128
    return cfg


def make_plan(edge_index, cfg, gather="indirect"):
    """Host-side graph preprocessing. Returns per-core meta arrays + C."""
    N, NCORES, NPC, NB = cfg["N"], cfg["NCORES"], cfg["NPC"], cfg["NB"]
    src = np.asarray(edge_index[0], dtype=np.int64)
    dst = np.asarray(edge_index[1], dtype=np.int64)
    loops = np.arange(N, dtype=np.int64)
    src_all = np.concatenate([src, loops])
    dst_all = np.concatenate([dst, loops])

    deg = np.bincount(dst_all, minlength=N).astype(np.float32)
    dinv = (1.0 / np.sqrt(deg)).astype(np.float32)
    norm = (dinv[src_all] * dinv[dst_all]).astype(np.float32)

    order = np.argsort(dst_all, kind="stable")
    ss, dd, nn = src_all[order], dst_all[order], norm[order]

    bounds = np.searchsorted(dd, np.arange(NCORES + 1) * NPC)

    if gather == "bulk":
        BR = -(-N // 4)  # bank rows
        assert BR <= 32767
        # first pass: SEGC = global max chunks per (block, bank) segment
        SEGC = 0
        percore = []
        for c in range(NCORES):
            lo, hi = bounds[c], bounds[c + 1]
            dloc = dd[lo:hi] - c * NPC
            blk = (dloc // 128).astype(np.int64)
            bank = (ss[lo:hi] // BR).astype(np.int64)
            key = blk * 4 + bank
            cnt = np.bincount(key, minlength=NB * 4)
            SEGC = max(SEGC, int(np.ceil(cnt.max() / 128)))
            percore.append((lo, hi, dloc, blk, bank, key, cnt))
        C = 4 * SEGC
        TOTC = NB * C
        idx16 = np.zeros((NCORES, 128, NB * C * 8), dtype=np.int16)
        dstL = np.full((NCORES, 128, TOTC), -1.0, dtype=np.float32)
        nrmE = np.zeros((NCORES, 128, TOTC), dtype=np.float32)
        for c in range(NCORES):
            lo, hi, dloc, blk, bank, key, cnt = percore[c]
            starts = np.zeros(NB * 4, dtype=np.int64)
            starts[1:] = np.cumsum(cnt)[:-1]
            # order edges by (block, bank) stably
            order2 = np.argsort(key, kind="stable")
            kk = key[order2]
            pos = np.arange(hi - lo) - starts[kk]
            b2, q2 = kk // 4, kk % 4
            col = b2 * C + q2 * SEGC + pos // 128
            p = pos % 128
            dstL[c, p, col] = (dloc[order2] % 128).astype(np.float32)
            nrmE[c, p, col] = nn[lo:hi][order2]
            # idx16 wrap: within (b,q) instruction flat i = pos;
            # cell = [i%16 (+16r), (b*C + q*SEGC)*8 + i//16]
            inbank = (ss[lo:hi][order2] - q2 * BR).astype(np.int16)
            c16 = (b2 * C + q2 * SEGC) * 8 + pos // 16
            p16 = pos % 16
            for r in range(8):
                idx16[c, p16 + 16 * r, c16] = inbank
        return dict(C=C, SEGC=SEGC, BR=BR, TOTC=TOTC, gather="bulk",
                    idx16=idx16, dstL=dstL, nrmE=nrmE)

    # first pass: C = global max chunks per block
    C = 0
    percore = []
    for c in range(NCORES):
        lo, hi = bounds[c], bounds[c + 1]
        dloc = dd[lo:hi] - c * NPC
        blk = (dloc // 128).astype(np.int64)
        cnt = np.bincount(blk, minlength=NB)
        C = max(C, int(np.ceil(cnt.max() / 128)))
        percore.append((lo, hi, dloc, blk, cnt))
    TOTC = NB * C

    srcI = np.zeros((NCORES, 128, TOTC), dtype=np.int32)
    dstL = np.full((NCORES, 128, TOTC), -1.0, dtype=np.float32)
    nrmE = np.zeros((NCORES, 128, TOTC), dtype=np.float32)
    for c in range(NCORES):
        lo, hi, dloc, blk, cnt = percore[c]
        starts = np.zeros(NB, dtype=np.int64)
        starts[1:] = np.cumsum(cnt)[:-1]
        pos = np.arange(hi - lo) - starts[blk]
        col = blk * C + pos // 128
        p = pos % 128
        srcI[c, p, col] = ss[lo:hi]
        dstL[c, p, col] = (dloc % 128).astype(np.float32)
        nrmE[c, p, col] = nn[lo:hi]
    return dict(C=C, TOTC=TOTC, gather="indirect",
                srcI=srcI, dstL=dstL, nrmE=nrmE)


def build_gcn(cfg, C, opts=None):
    opts = {
        "phases": "ABC",      # which phases to emit (prefix: A, AB, ABC)
        "bounds_skip": False,  # pad edges get huge src + bounds check to skip
        "swdge_queues": 1,    # spread gathers over N SWDGE queues
        "gather": "indirect",  # or "bulk" (banked int16 dma_gather)
        "SEGC": None, "BR": None,  # bulk params from plan
        "gbufs": None, "psbufs": 2, "sbufs": 8,  # pipeline depth knobs
    } | (opts or {})
    N, NCORES = cfg["N"], cfg["NCORES"]
    IN_DIM, HID, MID, OUT = cfg["IN_DIM"], cfg["HID"], cfg["MID"], cfg["OUT"]
    NPC, NB = cfg["NPC"], cfg["NB"]
    TOTC = NB * C
    NPAD = NB * 128
    KT = [(k, min(128, IN_DIM - k)) for k in range(0, IN_DIM, 128)]

    nc = bacc.Bacc(None, num_swdge_queues=opts["swdge_queues"])
    xT = nc.dram_tensor("xT", [IN_DIM, NPAD], F32, kind="ExternalInput")
    if opts["gather"] == "bulk":
        idx16 = nc.dram_tensor("idx16", [128, TOTC * 8], mybir.dt.int16,
                               kind="ExternalInput")
    else:
        srcI = nc.dram_tensor("srcI", [128, TOTC], I32, kind="ExternalInput")
    dstL = nc.dram_tensor("dstL", [128, TOTC], F32, kind="ExternalInput")
    nrmE = nc.dram_tensor("nrmE", [128, TOTC], F32, kind="ExternalInput")
    iota = nc.dram_tensor("iota", [128, 128], F32, kind="ExternalInput")
    w1 = nc.dram_tensor("w1", [IN_DIM, HID], F32, kind="ExternalInput")
    w2 = nc.dram_tensor("w2", [HID, HID], F32, kind="ExternalInput")
    wm1 = nc.dram_tensor("wm1", [HID, MID], F32, kind="ExternalInput")
    wm2 = nc.dram_tensor("wm2", [MID, OUT], F32, kind="ExternalInput")
    b1c = nc.dram_tensor("b1c", [HID, 1], F32, kind="ExternalInput")
    b2c = nc.dram_tensor("b2c", [HID, 1], F32, kind="ExternalInput")
    bm1c = nc.dram_tensor("bm1c", [MID, 1], F32, kind="ExternalInput")
    bm2c = nc.dram_tensor("bm2c", [OUT, 1], F32, kind="ExternalInput")
    outT = nc.dram_tensor("outT", [OUT, NPC], F32, kind="ExternalOutput")

    m0_loc = nc.dram_tensor("m0_loc", [NPC, HID], F32)
    m0_full = nc.dram_tensor("m0_full", [N, HID], F32, addr_space="Shared")
    m1_loc = nc.dram_tensor("m1_loc", [NPC, HID], F32)
    m1_full = nc.dram_tensor("m1_full", [N, HID], F32, addr_space="Shared")
    cc_sem = nc.alloc_semaphore(name="cc_sem")

    # ---------------- phase A: h0 = x @ W1 (node slice) ----------------
    with tile.TileContext(nc) as tc:
        with (
            tc.tile_pool(name="w1p", bufs=1) as w1p,
            tc.tile_pool(name="xtp", bufs=3) as xtp,
            tc.tile_pool(name="h0p", bufs=3) as h0p,
            tc.tile_pool(name="psA", bufs=2, space="PSUM") as psA,
        ):
            w1_sb = []
            for k0, ksz in KT:
                t = w1p.tile([ksz, HID], F32, tag=f"w1_{k0}")
                nc.sync.dma_start(out=t[:], in_=w1[k0:k0 + ksz, :])
                w1_sb.append(t)
            # node superchunks of up to 512
            n0 = 0
            while n0 < NPAD:
                W = min(512, NPAD - n0)
                xts = []
                for ti, (k0, ksz) in enumerate(KT):
                    t = xtp.tile([ksz, 512], F32, tag=f"xt_{ti}")
                    nc.sync.dma_start(out=t[:, :W], in_=xT[k0:k0 + ksz, n0:n0 + W])
                    xts.append(t)
                for sub in range(0, W, 128):
                    ps = psA.tile([128, HID], F32, space="PSUM")
                    for ti in range(len(KT)):
                        nc.tensor.matmul(
                            out=ps[:],
                            lhsT=xts[ti][:, sub:sub + 128],
                            rhs=w1_sb[ti][:],
                            start=(ti == 0), stop=(ti == len(KT) - 1),
                        )
                    h0 = h0p.tile([128, HID], F32, tag="h0")
                    nc.scalar.activation(
                        out=h0[:], in_=ps[:],
                        func=mybir.ActivationFunctionType.Copy)
                    rows = min(128, max(0, NPC - (n0 + sub)))
                    if rows > 0:
                        nc.sync.dma_start(
                            out=m0_loc[n0 + sub:n0 + sub + rows, :],
                            in_=h0[:rows, :])
                n0 += W

    nc.gpsimd.collective_compute(
        "AllGather", mybir.AluOpType.bypass,
        replica_groups=[list(range(NCORES))],
        ins=[m0_loc.ap().opt()], outs=[m0_full.ap().opt()],
    ).then_inc(cc_sem, 1)
    for eng in (nc.sync, nc.scalar, nc.vector, nc.tensor, nc.gpsimd):
        eng.wait_ge(cc_sem, 1)

    # ---------------- phase B: layer-1 aggregation + @W2 ----------------
    def aggregation_phase(tc, pools, table, meta, emit_block_tail):
        """Shared chunk loop: per dst block accumulate [feat,dst] in PSUM."""
        (gpool, spool, psB) = pools
        (srcI_sb, dstL_sb, nrmE_sb, iota_sb) = meta
        nq = opts["swdge_queues"]
        bulk = opts["gather"] == "bulk"
        SEGC, BR = opts["SEGC"], opts["BR"]
        for b in range(NB):
            acc = psB.tile([HID, 128], F32, space="PSUM")
            if bulk:
                gblk = gpool.tile([128, C, HID], F32, tag="g")
                for q in range(4):
                    lo, hi_ = q * BR, min((q + 1) * BR, N)
                    c16 = (b * C + q * SEGC) * 8
                    nc.gpsimd.dma_gather(
                        out_ap=gblk[:, q * SEGC:(q + 1) * SEGC, :],
                        in_ap=table.ap()[lo:hi_, :],
                        idxs_ap=srcI_sb[:, c16:c16 + SEGC * 8],
                        num_idxs=SEGC * 128,
                        num_idxs_reg=SEGC * 128,
                        elem_size=HID,
                        single_packet=False,
                        queue_num=q % nq,
                    )
            for j in range(C):
                col = b * C + j
                if bulk:
                    g = gblk[:, j, :]
                else:
                    gt = gpool.tile([128, HID], F32, tag="g")
                    kw = {}
                    if opts["bounds_skip"]:
                        kw = dict(bounds_check=N - 1, oob_is_err=False)
                    gi = nc.gpsimd.indirect_dma_start(
                        out=gt[:], out_offset=None,
                        in_=table.ap(),
                        in_offset=bass.IndirectOffsetOnAxis(
                            ap=srcI_sb[:, col:col + 1], axis=0),
                        **kw,
                    )
                    if nq > 1:
                        q = col % nq
                        if q:
                            gi.ins.queue = f"qPoolDynamic{q}"
                    g = gt[:]
                s = spool.tile([128, 128], F32, tag="s")
                nc.vector.tensor_scalar(
                    out=s[:], in0=iota_sb[:],
                    scalar1=dstL_sb[:, col:col + 1],
                    scalar2=nrmE_sb[:, col:col + 1],
                    op0=mybir.AluOpType.is_equal,
                    op1=mybir.AluOpType.mult,
                )
                nc.tensor.matmul(
                    out=acc[:], lhsT=g, rhs=s[:],
                    start=(j == 0), stop=(j == C - 1),
                )
            emit_block_tail(b, acc)

    if "B" not in opts["phases"]:
        nc.compile()
        return nc

    def load_meta(pool):
        if opts["gather"] == "bulk":
            srcI_sb = pool.tile([128, TOTC * 8], mybir.dt.int16)
            nc.sync.dma_start(out=srcI_sb[:], in_=idx16[:])
        else:
            srcI_sb = pool.tile([128, TOTC], I32)
            nc.sync.dma_start(out=srcI_sb[:], in_=srcI[:])
        dstL_sb = pool.tile([128, TOTC], F32)
        nc.sync.dma_start(out=dstL_sb[:], in_=dstL[:])
        nrmE_sb = pool.tile([128, TOTC], F32)
        nc.sync.dma_start(out=nrmE_sb[:], in_=nrmE[:])
        iota_sb = pool.tile([128, 128], F32)
        nc.sync.dma_start(out=iota_sb[:], in_=iota[:])
        return srcI_sb, dstL_sb, nrmE_sb, iota_sb

    with tile.TileContext(nc) as tc:
        with (
            tc.tile_pool(name="metaB", bufs=1) as metaB,
            tc.tile_pool(name="gB", bufs=opts["gbufs"] or (3 if opts["gather"] == "bulk" else 12)) as gB,
            tc.tile_pool(name="sB", bufs=opts["sbufs"]) as sB,
            tc.tile_pool(name="psB", bufs=opts["psbufs"], space="PSUM") as psB,
            tc.tile_pool(name="psB2", bufs=2, space="PSUM") as psB2,
            tc.tile_pool(name="hB", bufs=3) as hB,
            tc.tile_pool(name="mB", bufs=3) as mB,
            tc.tile_pool(name="wB", bufs=1) as wB,
        ):
            meta = load_meta(metaB)
            w2_sb = wB.tile([HID, HID], F32)
            nc.sync.dma_start(out=w2_sb[:], in_=w2[:])
            b1_sb = wB.tile([HID, 1], F32)
            nc.sync.dma_start(out=b1_sb[:], in_=b1c[:])

            def tail_B(b, acc):
                h1T = hB.tile([HID, 128], F32, tag="h1T")
                nc.vector.tensor_scalar(
                    out=h1T[:], in0=acc[:],
                    scalar1=b1_sb[:], scalar2=None,
                    op0=mybir.AluOpType.add,
                )
                m1ps = psB2.tile([128, HID], F32, space="PSUM")
                nc.tensor.matmul(out=m1ps[:], lhsT=h1T[:], rhs=w2_sb[:],
                                 start=True, stop=True)
                m1sb = mB.tile([128, HID], F32, tag="m1sb")
                nc.scalar.activation(out=m1sb[:], in_=m1ps[:],
                                     func=mybir.ActivationFunctionType.Copy)
                rows = min(128, NPC - b * 128)
                nc.sync.dma_start(out=m1_loc[b * 128:b * 128 + rows, :],
                                  in_=m1sb[:rows, :])

            aggregation_phase(tc, (gB, sB, psB), m0_full, meta, tail_B)

    nc.gpsimd.collective_compute(
        "AllGather", mybir.AluOpType.bypass,
        replica_groups=[list(range(NCORES))],
        ins=[m1_loc.ap().opt()], outs=[m1_full.ap().opt()],
    ).then_inc(cc_sem, 1)
    for eng in (nc.sync, nc.scalar, nc.vector, nc.tensor, nc.gpsimd):
        eng.wait_ge(cc_sem, 2)

    # ---------------- phase C: layer-2 aggregation + MLP head ----------------
    if "C" not in opts["phases"]:
        nc.compile()
        return nc
    with tile.TileContext(nc) as tc:
        with (
            tc.tile_pool(name="metaC", bufs=1) as metaC,
            tc.tile_pool(name="gC", bufs=opts["gbufs"] or (3 if opts["gather"] == "bulk" else 12)) as gC,
            tc.tile_pool(name="sC", bufs=opts["sbufs"]) as sC,
            tc.tile_pool(name="psC", bufs=opts["psbufs"], space="PSUM") as psC,
            tc.tile_pool(name="psC2", bufs=2, space="PSUM") as psC2,
            tc.tile_pool(name="psC3", bufs=2, space="PSUM") as psC3,
            tc.tile_pool(name="hC", bufs=3) as hC,
            tc.tile_pool(name="wC", bufs=1) as wC,
        ):
            meta = load_meta(metaC)
            wm1_sb = wC.tile([HID, MID], F32)
            nc.sync.dma_start(out=wm1_sb[:], in_=wm1[:])
            wm2_sb = wC.tile([MID, OUT], F32)
            nc.sync.dma_start(out=wm2_sb[:], in_=wm2[:])
            b2_sb = wC.tile([HID, 1], F32)
            nc.sync.dma_start(out=b2_sb[:], in_=b2c[:])
            bm1_sb = wC.tile([MID, 1], F32)
            nc.sync.dma_start(out=bm1_sb[:], in_=bm1c[:])
            bm2_sb = wC.tile([OUT, 1], F32)
            nc.sync.dma_start(out=bm2_sb[:], in_=bm2c[:])

            def tail_C(b, acc):
                r2T = hC.tile([HID, 128], F32, tag="r2T")
                nc.scalar.activation(
                    out=r2T[:], in_=acc[:],
                    func=mybir.ActivationFunctionType.Relu, bias=b2_sb[:])
                y1 = psC2.tile([MID, 128], F32, space="PSUM")
                nc.tensor.matmul(out=y1[:], lhsT=wm1_sb[:], rhs=r2T[:],
                                 start=True, stop=True)
                r1T = hC.tile([MID, 128], F32, tag="r1T")
                nc.scalar.activation(
                    out=r1T[:], in_=y1[:],
                    func=mybir.ActivationFunctionType.Relu, bias=bm1_sb[:])
                z = psC3.tile([OUT, 128], F32, space="PSUM")
                nc.tensor.matmul(out=z[:], lhsT=wm2_sb[:], rhs=r1T[:],
                                 start=True, stop=True)
                zb = hC.tile([OUT, 128], F32, tag="zb")
                nc.vector.tensor_scalar(
                    out=zb[:], in0=z[:], scalar1=bm2_sb[:], scalar2=None,
                    op0=mybir.AluOpType.add)
                rows = min(128, NPC - b * 128)
                nc.sync.dma_start(out=outT[:, b * 128:b * 128 + rows],
                                  in_=zb[:, :rows])

            aggregation_phase(tc, (gC, sC, psC), m1_full, meta, tail_C)

    nc.compile()
    return nc


def make_inmaps(cfg, plan, inputs):
    """Build per-core input maps from full problem inputs + plan."""
    N, NCORES, NPC, NB = cfg["N"], cfg["NCORES"], cfg["NPC"], cfg["NB"]
    IN_DIM, OUT = cfg["IN_DIM"], cfg["OUT"]
    NPAD = NB * 128
    x = np.asarray(inputs["x"], dtype=np.float32)
    iota = np.broadcast_to(
        np.arange(128, dtype=np.float32), (128, 128)).copy()
    w1 = np.asarray(inputs["w1"], np.float32)
    w2 = np.asarray(inputs["w2"], np.float32)
    wm1 = np.asarray(inputs["wm1"], np.float32)
    wm2 = np.asarray(inputs["wm2"], np.float32)
    b1c = np.asarray(inputs["b1"], np.float32)[:, None]
    b2c = np.asarray(inputs["b2"], np.float32)[:, None]
    bm1c = np.asarray(inputs["bm1"], np.float32)[:, None]
    bm2c = np.asarray(inputs["bm2"], np.float32)[:, None]
    in_maps = []
    for c in range(NCORES):
        xTc = np.zeros((IN_DIM, NPAD), dtype=np.float32)
        xTc[:, :NPC] = x[c * NPC:(c + 1) * NPC].T
        m = ({"idx16": plan["idx16"][c]} if plan.get("gather") == "bulk"
             else {"srcI": plan["srcI"][c]})
        in_maps.append({
            "xT": xTc, **m,
            "dstL": plan["dstL"][c],
            "nrmE": plan["nrmE"][c], "iota": iota,
            "w1": w1, "w2": w2, "wm1": wm1, "wm2": wm2,
            "b1c": b1c, "b2c": b2c, "bm1c": bm1c, "bm2c": bm2c,
        })
    return in_maps


GATHER = "bulk"       # banked int16 dma_gather backend ("indirect" fallback)
QUEUES = 4            # spread bank gathers over 4 SWDGE queues
BOUNDS_SKIP = False   # indirect only: pad edges skipped via bounds check


def kernel(**inputs):
    """Full-input entry point: returns [N, 4] float32."""
    cfg = derive(default_cfg())
    plan = make_plan(np.asarray(inputs["edge_index"]), cfg, gather=GATHER)
    bounds = BOUNDS_SKIP and plan.get("gather") != "bulk"
    if bounds:
        plan["srcI"][plan["dstL"] < 0] = 2**30
    opts = {"gather": plan.get("gather", "indirect"),
            "SEGC": plan.get("SEGC"), "BR": plan.get("BR"),
            "swdge_queues": QUEUES, "bounds_skip": bounds,
            "gbufs": 6, "psbufs": 3, "sbufs": 12}
    nc = build_gcn(cfg, plan["C"], opts)
    in_maps = make_inmaps(cfg, plan, inputs)
    from concourse.bass_utils import run_bass_kernel_spmd
    res = run_bass_kernel_spmd(
        nc, in_maps, core_ids=list(range(cfg["NCORES"])))
    return np.concatenate([res.results[c]["outT"].T
                           for c in range(cfg["NCORES"])], axis=0)



# revision 8
# speedup vs baseline: 2906.5259x; 2906.5259x over previous
"""Trainium2 Bass kernel for a 2-layer GCN (GCNConv x2 + MLP head),
8-core SPMD via run_bass_kernel_spmd.

Entry point: kernel(**inputs) -> np.ndarray [100000, 4] float32.

v2 design (bf16 everywhere except accumulation):
- Nodes are partitioned across 8 cores (12500 each). Each core projects
  its slice (x @ W1 in bf16), slices are AllGathered (bf16) into a full
  [N, 128] bf16 gather table per core.
- Edges (+ self-loops) are partitioned by destination. Per core they are
  bucketed by (dst block of 128, src bank of 25000). Each (block, bank)
  segment is padded to a multiple of 128 "slots"; segments of G=8
  consecutive blocks in the same bank form one dma_gather instruction
  (~4-6K indices), spread over 4 SWDGE queues (queue = bank).
- Per destination block the gathered chunks are reduced with a
  selection-matrix matmul into a [HID, 128] PSUM accumulator:
  s[e, d] = (iota==dstloc_e) * norm_e is built in 2 batched DVE
  tensor_tensor passes per block (broadcast APs over all of the block's
  chunks), then one matmul per chunk accumulates feat x dst.
- Layer-1 tail fuses +b1 and @W2; layer-2 tail fuses the MLP head.
  Chunk counts per (block, bank) are maxed across cores so all 8 cores
  run an identical program; per-core pads carry weight 0.
"""

import numpy as np

import concourse.bacc as bacc
import concourse.mybir as mybir
import concourse.tile as tile

F32 = mybir.dt.float32
BF16 = mybir.dt.bfloat16
I16 = mybir.dt.int16

N, IN_DIM, HID, MID, OUT = 100000, 518, 128, 64, 4
NCORES = 8
NPC = N // NCORES            # 12500
NB = (NPC + 127) // 128      # 98
NPAD = NB * 128              # 12544
NBANK = 4
BR = N // NBANK              # 25000 rows per bank (int16-indexable)
G = 8                        # dst blocks per gather group
NG = (NB + G - 1) // G       # 13
KT = [(k, min(128, IN_DIM - k)) for k in range(0, IN_DIM, 128)]


def default_cfg():
    return dict(N=N, NCORES=NCORES, IN_DIM=IN_DIM, HID=HID, MID=MID, OUT=OUT)


def derive(cfg):
    cfg = dict(cfg)
    cfg["NPC"] = NPC
    cfg["NB"] = NB
    return cfg


def _group_blocks(g):
    return range(g * G, min((g + 1) * G, NB))


def make_plan(edge_index):
    """Host-side graph preprocessing.

    Returns dict with the static chunk structure (shared by all cores)
    and per-core idx16/dstL/nrmE arrays.
    """
    src = np.asarray(edge_index[0], dtype=np.int64)
    dst = np.asarray(edge_index[1], dtype=np.int64)
    loops = np.arange(N, dtype=np.int64)
    ss_all = np.concatenate([src, loops])
    dd_all = np.concatenate([dst, loops])

    deg = np.bincount(dd_all, minlength=N).astype(np.float64)
    dinv = 1.0 / np.sqrt(deg)
    nn_all = (dinv[ss_all] * dinv[dd_all]).astype(np.float32)

    order = np.argsort(dd_all, kind="stable")
    ss, dd, nn = ss_all[order], dd_all[order], nn_all[order]
    bounds = np.searchsorted(dd, np.arange(NCORES + 1) * NPC)

    percore = []
    cnts = np.zeros((NCORES, NB * NBANK), dtype=np.int64)
    for c in range(NCORES):
        lo, hi = bounds[c], bounds[c + 1]
        dloc = dd[lo:hi] - c * NPC
        blk = dloc >> 7
        bank = ss[lo:hi] % NBANK      # mod-interleaved banks (spreads
        key = blk * NBANK + bank      # self-loops evenly)
        cnts[c] = np.bincount(key, minlength=NB * NBANK)
        percore.append((lo, hi, dloc, key))

    # static chunk structure: per (block, bank) number of 128-slot chunks
    nch_bq = np.ceil(cnts.max(axis=0) / 128).astype(np.int64)  # [NB*4]
    nch_bq_2d = nch_bq.reshape(NB, NBANK)
    n_b = nch_bq_2d.sum(axis=1)                  # chunks per block
    chunk_base_b = np.zeros(NB + 1, dtype=np.int64)
    chunk_base_b[1:] = np.cumsum(n_b)
    TCH = int(chunk_base_b[-1])                  # total chunks (per layer)

    # gather instruction layout: ordered (g, q); within: blocks asc
    S_gq = np.zeros((NG, NBANK), dtype=np.int64)     # slots per instruction
    seg_choff = np.zeros((NB, NBANK), dtype=np.int64)  # chunk offset of
    for g in range(NG):                                # block b in instr (g,q)
        for q in range(NBANK):
            off = 0
            for b in _group_blocks(g):
                seg_choff[b, q] = off
                off += nch_bq_2d[b, q]
            S_gq[g, q] = off * 128
    inst_slotbase = np.zeros((NG, NBANK + 1), dtype=np.int64)
    flat = S_gq.reshape(-1)
    starts = np.zeros(NG * NBANK, dtype=np.int64)
    starts[1:] = np.cumsum(flat)[:-1]
    TOTSLOT = int(flat.sum())                    # = 128 * TCH

    # per-edge static targets (same formulas for every core, data varies)
    # meta col of chunk (b, q, j): chunk_base_b[b] + qoff_bq + j
    qoff_bq = np.zeros((NB, NBANK), dtype=np.int64)
    qoff_bq[:, 1:] = np.cumsum(nch_bq_2d, axis=1)[:, :-1]
    # gather slot base of segment (b,q): starts[(g*NBANK)+q] + seg_choff*128
    gof = np.zeros((NB, NBANK), dtype=np.int64)
    for b in range(NB):
        g = b // G
        for q in range(NBANK):
            gof[b, q] = starts[g * NBANK + q] + seg_choff[b, q] * 128

    idx16 = np.zeros((NCORES, 128, TOTSLOT // 16), dtype=np.int16)
    dstL = np.full((NCORES, 128, TCH), -1.0, dtype=np.float32)
    nrmE = np.zeros((NCORES, 128, TCH), dtype=np.float32)
    for c in range(NCORES):
        lo, hi, dloc, key = percore[c]
        order2 = np.argsort(key, kind="stable")
        kk = key[order2]
        seg_starts = np.zeros(NB * NBANK, dtype=np.int64)
        seg_starts[1:] = np.cumsum(cnts[c])[:-1]
        pos = np.arange(hi - lo) - seg_starts[kk]
        b2, q2 = kk // NBANK, kk % NBANK
        j = pos >> 7
        p = pos & 127
        t = chunk_base_b[b2] + qoff_bq[b2, q2] + j
        dstL[c, p, t] = (dloc[order2] & 127).astype(np.float32)
        nrmE[c, p, t] = nn[lo:hi][order2]
        s_glob = gof[b2, q2] + pos
        inbank = (ss[lo:hi][order2] // NBANK).astype(np.int16)
        cc = s_glob >> 4
        pp = s_glob & 15
        for r in range(8):
            idx16[c, pp + 16 * r, cc] = inbank
    dt_bf16 = mybir.dt.np(BF16)
    return dict(TCH=TCH, TOTSLOT=TOTSLOT, nch_bq=nch_bq_2d, n_b=n_b,
                chunk_base_b=chunk_base_b, qoff_bq=qoff_bq,
                seg_choff=seg_choff, S_gq=S_gq, starts=starts,
                idx16=idx16, dstL=dstL.astype(dt_bf16),
                nrmE=nrmE.astype(dt_bf16))


def build_gcn(plan):
    TCH, TOTSLOT = plan["TCH"], plan["TOTSLOT"]
    nch_bq = plan["nch_bq"]
    n_b = plan["n_b"]
    chunk_base_b = plan["chunk_base_b"]
    qoff_bq = plan["qoff_bq"]
    seg_choff = plan["seg_choff"]
    S_gq = plan["S_gq"]
    starts = plan["starts"]

    nc = bacc.Bacc(None, num_swdge_queues=NBANK)
    xT = nc.dram_tensor("xT", [IN_DIM, NPAD], BF16, kind="ExternalInput")
    idx16 = nc.dram_tensor("idx16", [128, TOTSLOT // 16], I16,
                           kind="ExternalInput")
    dstL = nc.dram_tensor("dstL", [128, TCH], BF16, kind="ExternalInput")
    nrmE = nc.dram_tensor("nrmE", [128, TCH], BF16, kind="ExternalInput")
    iota = nc.dram_tensor("iota", [128, 128], BF16, kind="ExternalInput")
    w1 = nc.dram_tensor("w1", [IN_DIM, HID], BF16, kind="ExternalInput")
    w2 = nc.dram_tensor("w2", [HID, HID], BF16, kind="ExternalInput")
    wm1 = nc.dram_tensor("wm1", [HID, MID], BF16, kind="ExternalInput")
    wm2 = nc.dram_tensor("wm2", [MID, OUT], BF16, kind="ExternalInput")
    b1c = nc.dram_tensor("b1c", [HID, 1], F32, kind="ExternalInput")
    b2c = nc.dram_tensor("b2c", [HID, 1], F32, kind="ExternalInput")
    bm1c = nc.dram_tensor("bm1c", [MID, 1], F32, kind="ExternalInput")
    bm2c = nc.dram_tensor("bm2c", [OUT, 1], F32, kind="ExternalInput")
    outT = nc.dram_tensor("outT", [OUT, NPC], F32, kind="ExternalOutput")

    m0_loc = nc.dram_tensor("m0_loc", [NPC, HID], BF16)
    m0_full = nc.dram_tensor("m0_full", [N, HID], BF16, addr_space="Shared")
    m1_loc = nc.dram_tensor("m1_loc", [NPC, HID], BF16)
    m1_full = nc.dram_tensor("m1_full", [N, HID], BF16, addr_space="Shared")
    cc_sem = nc.alloc_semaphore(name="cc_sem")

    # ---------------- phase A: h0 = x @ W1 (node slice) ----------------
    with tile.TileContext(nc) as tc:
        with (
            tc.tile_pool(name="w1p", bufs=1) as w1p,
            tc.tile_pool(name="xtp", bufs=3) as xtp,
            tc.tile_pool(name="h0p", bufs=3) as h0p,
            tc.tile_pool(name="psA", bufs=3, space="PSUM") as psA,
        ):
            w1_sb = []
            for k0, ksz in KT:
                t = w1p.tile([ksz, HID], BF16, tag=f"w1_{k0}")
                nc.sync.dma_start(out=t[:], in_=w1[k0:k0 + ksz, :])
                w1_sb.append(t)
            n0 = 0
            while n0 < NPAD:
                W = min(512, NPAD - n0)
                xts = []
                for ti, (k0, ksz) in enumerate(KT):
                    t = xtp.tile([ksz, 512], BF16, tag=f"xt_{ti}")
                    nc.sync.dma_start(out=t[:, :W], in_=xT[k0:k0 + ksz, n0:n0 + W])
                    xts.append(t)
                for sub in range(0, W, 128):
                    ps = psA.tile([128, HID], F32, space="PSUM")
                    for ti in range(len(KT)):
                        nc.tensor.matmul(
                            out=ps[:],
                            lhsT=xts[ti][:, sub:sub + 128],
                            rhs=w1_sb[ti][:],
                            start=(ti == 0), stop=(ti == len(KT) - 1),
                        )
                    h0 = h0p.tile([128, HID], BF16, tag="h0")
                    nc.scalar.activation(
                        out=h0[:], in_=ps[:],
                        func=mybir.ActivationFunctionType.Copy)
                    rows = min(128, max(0, NPC - (n0 + sub)))
                    if rows > 0:
                        nc.sync.dma_start(
                            out=m0_loc[n0 + sub:n0 + sub + rows, :],
                            in_=h0[:rows, :])
                n0 += W

    nc.gpsimd.collective_compute(
        "AllGather", mybir.AluOpType.bypass,
        replica_groups=[list(range(NCORES))],
        ins=[m0_loc.ap().opt()], outs=[m0_full.ap().opt()],
    ).then_inc(cc_sem, 1)
    for eng in (nc.sync, nc.scalar, nc.vector, nc.tensor, nc.gpsimd):
        eng.wait_ge(cc_sem, 1)

    # ------------- aggregation sweep (shared by both layers) -------------
    def sweep(tc, pools, table, meta, tail):
        gq, sP, eP, psAcc = pools
        idx_sb, dstL_sb, nrmE_sb, iota_sb = meta
        gt = {}
        for g in range(NG):
            for q in range(NBANK):
                slots = int(S_gq[g, q])
                if slots == 0:
                    continue
                t = gq.tile([128, slots // 128, HID], BF16, tag=f"g{q}")
                base = int(starts[g * NBANK + q])
                nc.gpsimd.dma_gather(
                    out_ap=t[:, :, :],
                    in_ap=table.ap()[q::NBANK, :],
                    idxs_ap=idx_sb[:, base // 16:(base + slots) // 16],
                    num_idxs=slots,
                    num_idxs_reg=slots,
                    elem_size=HID,
                    elem_step=HID * NBANK,
                    single_packet=False,
                    queue_num=q,
                )
                gt[g, q] = t
            for b in _group_blocks(g):
                nb = int(n_b[b])
                t0 = int(chunk_base_b[b])
                eq = eP.tile([128, nb, 128], BF16, tag="eq")
                s_b = sP.tile([128, nb, 128], BF16, tag="s")
                iota_bc = iota_sb[:, :].unsqueeze(1).broadcast_to(
                    [128, nb, 128])
                nc.vector.tensor_tensor(
                    out=eq[:, :, :], in0=iota_bc,
                    in1=dstL_sb[:, t0:t0 + nb].unsqueeze(2).broadcast_to(
                        [128, nb, 128]),
                    op=mybir.AluOpType.is_equal)
                nc.vector.tensor_tensor(
                    out=s_b[:, :, :], in0=eq[:, :, :],
                    in1=nrmE_sb[:, t0:t0 + nb].unsqueeze(2).broadcast_to(
                        [128, nb, 128]),
                    op=mybir.AluOpType.mult)
                acc = psAcc.tile([HID, 128], F32, space="PSUM")
                mm, nmm = 0, nb
                for q in range(NBANK):
                    for j in range(int(nch_bq[b, q])):
                        nc.tensor.matmul(
                            out=acc[:],
                            lhsT=gt[b // G, q][:, int(seg_choff[b, q]) + j, :],
                            rhs=s_b[:, int(qoff_bq[b, q]) + j, :],
                            start=(mm == 0), stop=(mm == nmm - 1),
                        )
                        mm += 1
                tail(b, acc)

    def load_meta(pool):
        idx_sb = pool.tile([128, TOTSLOT // 16], I16)
        nc.sync.dma_start(out=idx_sb[:], in_=idx16[:])
        dstL_sb = pool.tile([128, TCH], BF16)
        nc.sync.dma_start(out=dstL_sb[:], in_=dstL[:])
        nrmE_sb = pool.tile([128, TCH], BF16)
        nc.sync.dma_start(out=nrmE_sb[:], in_=nrmE[:])
        iota_sb = pool.tile([128, 128], BF16)
        nc.sync.dma_start(out=iota_sb[:], in_=iota[:])
        return idx_sb, dstL_sb, nrmE_sb, iota_sb

    # ---------------- phase B: layer-1 aggregation + @W2 ----------------
    with tile.TileContext(nc) as tc:
        with (
            tc.tile_pool(name="metaB", bufs=1) as metaB,
            tc.tile_pool(name="gB", bufs=2) as gB,
            tc.tile_pool(name="sB", bufs=3) as sB,
            tc.tile_pool(name="eB", bufs=3) as eB,
            tc.tile_pool(name="psB", bufs=4, space="PSUM") as psB,
            tc.tile_pool(name="psB2", bufs=2, space="PSUM") as psB2,
            tc.tile_pool(name="hB", bufs=3) as hB,
            tc.tile_pool(name="wB", bufs=1) as wB,
        ):
            meta = load_meta(metaB)
            w2_sb = wB.tile([HID, HID], BF16)
            nc.sync.dma_start(out=w2_sb[:], in_=w2[:])
            b1_sb = wB.tile([HID, 1], F32)
            nc.sync.dma_start(out=b1_sb[:], in_=b1c[:])

            def tail_B(b, acc):
                h1T = hB.tile([HID, 128], BF16, tag="h1T")
                nc.vector.tensor_scalar(
                    out=h1T[:], in0=acc[:],
                    scalar1=b1_sb[:], scalar2=None,
                    op0=mybir.AluOpType.add,
                )
                m1ps = psB2.tile([128, HID], F32, space="PSUM")
                nc.tensor.matmul(out=m1ps[:], lhsT=h1T[:], rhs=w2_sb[:],
                                 start=True, stop=True)
                m1sb = hB.tile([128, HID], BF16, tag="m1sb")
                nc.scalar.activation(out=m1sb[:], in_=m1ps[:],
                                     func=mybir.ActivationFunctionType.Copy)
                rows = min(128, NPC - b * 128)
                nc.sync.dma_start(out=m1_loc[b * 128:b * 128 + rows, :],
                                  in_=m1sb[:rows, :])

            sweep(tc, (gB, sB, eB, psB), m0_full, meta, tail_B)

    nc.gpsimd.collective_compute(
        "AllGather", mybir.AluOpType.bypass,
        replica_groups=[list(range(NCORES))],
        ins=[m1_loc.ap().opt()], outs=[m1_full.ap().opt()],
    ).then_inc(cc_sem, 1)
    for eng in (nc.sync, nc.scalar, nc.vector, nc.tensor, nc.gpsimd):
        eng.wait_ge(cc_sem, 2)

    # ---------------- phase C: layer-2 aggregation + MLP head ------------
    with tile.TileContext(nc) as tc:
        with (
            tc.tile_pool(name="metaC", bufs=1) as metaC,
            tc.tile_pool(name="gC", bufs=2) as gC,
            tc.tile_pool(name="sC", bufs=3) as sC,
            tc.tile_pool(name="eC", bufs=3) as eC,
            tc.tile_pool(name="psC", bufs=4, space="PSUM") as psC,
            tc.tile_pool(name="psC2", bufs=2, space="PSUM") as psC2,
            tc.tile_pool(name="psC3", bufs=2, space="PSUM") as psC3,
            tc.tile_pool(name="hC", bufs=3) as hC,
            tc.tile_pool(name="wC", bufs=1) as wC,
        ):
            meta = load_meta(metaC)
            wm1_sb = wC.tile([HID, MID], BF16)
            nc.sync.dma_start(out=wm1_sb[:], in_=wm1[:])
            wm2_sb = wC.tile([MID, OUT], BF16)
            nc.sync.dma_start(out=wm2_sb[:], in_=wm2[:])
            b2_sb = wC.tile([HID, 1], F32)
            nc.sync.dma_start(out=b2_sb[:], in_=b2c[:])
            bm1_sb = wC.tile([MID, 1], F32)
            nc.sync.dma_start(out=bm1_sb[:], in_=bm1c[:])
            bm2_sb = wC.tile([OUT, 1], F32)
            nc.sync.dma_start(out=bm2_sb[:], in_=bm2c[:])

            def tail_C(b, acc):
                r2T = hC.tile([HID, 128], BF16, tag="r2T")
                nc.scalar.activation(
                    out=r2T[:], in_=acc[:],
                    func=mybir.ActivationFunctionType.Relu, bias=b2_sb[:])
                y1 = psC2.tile([MID, 128], F32, space="PSUM")
                nc.tensor.matmul(out=y1[:], lhsT=wm1_sb[:], rhs=r2T[:],
                                 start=True, stop=True)
                r1T = hC.tile([MID, 128], BF16, tag="r1T")
                nc.scalar.activation(
                    out=r1T[:], in_=y1[:],
                    func=mybir.ActivationFunctionType.Relu, bias=bm1_sb[:])
                z = psC3.tile([OUT, 128], F32, space="PSUM")
                nc.tensor.matmul(out=z[:], lhsT=wm2_sb[:], rhs=r1T[:],
                                 start=True, stop=True)
                zb = hC.tile([OUT, 128], F32, tag="zb")
                nc.vector.tensor_scalar(
                    out=zb[:], in0=z[:], scalar1=bm2_sb[:], scalar2=None,
                    op0=mybir.AluOpType.add)
                rows = min(128, NPC - b * 128)
                nc.sync.dma_start(out=outT[:, b * 128:b * 128 + rows],
                                  in_=zb[:, :rows])

            sweep(tc, (gC, sC, eC, psC), m1_full, meta, tail_C)

    nc.compile()
    return nc


def make_inmaps(plan, inputs):
    bf = mybir.dt.np(BF16)
    x = np.asarray(inputs["x"], dtype=np.float32)
    iota = np.broadcast_to(
        np.arange(128, dtype=np.float32), (128, 128)).astype(bf)
    w1 = np.asarray(inputs["w1"], np.float32).astype(bf)
    w2 = np.asarray(inputs["w2"], np.float32).astype(bf)
    wm1 = np.asarray(inputs["wm1"], np.float32).astype(bf)
    wm2 = np.asarray(inputs["wm2"], np.float32).astype(bf)
    b1c = np.asarray(inputs["b1"], np.float32)[:, None]
    b2c = np.asarray(inputs["b2"], np.float32)[:, None]
    bm1c = np.asarray(inputs["bm1"], np.float32)[:, None]
    bm2c = np.asarray(inputs["bm2"], np.float32)[:, None]
    in_maps = []
    for c in range(NCORES):
        xTc = np.zeros((IN_DIM, NPAD), dtype=bf)
        xTc[:, :NPC] = x[c * NPC:(c + 1) * NPC].T.astype(bf)
        in_maps.append({
            "xT": xTc, "idx16": plan["idx16"][c],
            "dstL": plan["dstL"][c], "nrmE": plan["nrmE"][c], "iota": iota,
            "w1": w1, "w2": w2, "wm1": wm1, "wm2": wm2,
            "b1c": b1c, "b2c": b2c, "bm1c": bm1c, "bm2c": bm2c,
        })
    return in_maps


def build(inputs):
    plan = make_plan(np.asarray(inputs["edge_index"]))
    nc = build_gcn(plan)
    in_maps = make_inmaps(plan, inputs)
    return nc, in_maps


def assemble(results, cfg=None):
    return np.concatenate(
        [np.asarray(results[c]["outT"], dtype=np.float32).T
         for c in range(NCORES)], axis=0)


def kernel(**inputs):
    """Full-input entry point: returns [N, 4] float32."""
    nc, in_maps = build(inputs)
    from concourse.bass_utils import run_bass_kernel_spmd
    res = run_bass_kernel_spmd(nc, in_maps, core_ids=list(range(NCORES)))
    return assemble(res.results)


# revision 19
# speedup vs baseline: 3450.3046x; 1.1871x over previous
"""Trainium2 Bass kernel for a 2-layer GCN (GCNConv x2 + MLP head),
8-core SPMD via run_bass_kernel_spmd.

Entry point: kernel(**inputs) -> np.ndarray [100000, 4] float32.

v2 design (bf16 everywhere except accumulation):
- Nodes are partitioned across 8 cores (12500 each). Each core projects
  its slice (x @ W1 in bf16), slices are AllGathered (bf16) into a full
  [N, 128] bf16 gather table per core.
- Edges (+ self-loops) are partitioned by destination. Per core they are
  bucketed by (dst block of 128, src bank of 25000). Each (block, bank)
  segment is padded to a multiple of 128 "slots"; segments of G=8
  consecutive blocks in the same bank form one dma_gather instruction
  (~4-6K indices), spread over 4 SWDGE queues (queue = bank).
- Per destination block the gathered chunks are reduced with a
  selection-matrix matmul into a [HID, 128] PSUM accumulator:
  s[e, d] = (iota==dstloc_e) * norm_e is built in 2 batched DVE
  tensor_tensor passes per block (broadcast APs over all of the block's
  chunks), then one matmul per chunk accumulates feat x dst.
- Layer-1 tail fuses +b1 and @W2; layer-2 tail fuses the MLP head.
  Chunk counts per (block, bank) are maxed across cores so all 8 cores
  run an identical program; per-core pads carry weight 0.
"""

import numpy as np

import concourse.bacc as bacc
import concourse.mybir as mybir
import concourse.tile as tile

F32 = mybir.dt.float32
BF16 = mybir.dt.bfloat16
I16 = mybir.dt.int16

N, IN_DIM, HID, MID, OUT = 100000, 518, 128, 64, 4
NCORES = 8
NPC = N // NCORES            # 12500
NB = (NPC + 127) // 128      # 98
NPAD = NB * 128              # 12544
NBANK = 4
BR = N // NBANK              # 25000 rows per bank (int16-indexable)
G = 8                        # dst blocks per gather group
NG = (NB + G - 1) // G       # 13
KT = [(k, min(128, IN_DIM - k)) for k in range(0, IN_DIM, 128)]


def default_cfg():
    return dict(N=N, NCORES=NCORES, IN_DIM=IN_DIM, HID=HID, MID=MID, OUT=OUT)


def derive(cfg):
    cfg = dict(cfg)
    cfg["NPC"] = NPC
    cfg["NB"] = NB
    return cfg


def _group_blocks(g):
    return range(g * G, min((g + 1) * G, NB))


def make_plan(edge_index):
    """Host-side graph preprocessing.

    Returns dict with the static chunk structure (shared by all cores)
    and per-core idx16/dstL/nrmE arrays.
    """
    src = np.asarray(edge_index[0], dtype=np.int64)
    dst = np.asarray(edge_index[1], dtype=np.int64)
    loops = np.arange(N, dtype=np.int64)
    ss_all = np.concatenate([src, loops])
    dd_all = np.concatenate([dst, loops])

    deg = np.bincount(dd_all, minlength=N).astype(np.float64)
    dinv = 1.0 / np.sqrt(deg)
    nn_all = (dinv[ss_all] * dinv[dd_all]).astype(np.float32)

    order = np.argsort(dd_all, kind="stable")
    ss, dd, nn = ss_all[order], dd_all[order], nn_all[order]
    bounds = np.searchsorted(dd, np.arange(NCORES + 1) * NPC)

    percore = []
    cnts = np.zeros((NCORES, NB * NBANK), dtype=np.int64)
    for c in range(NCORES):
        lo, hi = bounds[c], bounds[c + 1]
        dloc = dd[lo:hi] - c * NPC
        blk = dloc >> 7
        bank = ss[lo:hi] % NBANK      # mod-interleaved banks (spreads
        key = blk * NBANK + bank      # self-loops evenly)
        cnts[c] = np.bincount(key, minlength=NB * NBANK)
        percore.append((lo, hi, dloc, key))

    # static chunk structure: per (block, bank) number of 128-slot chunks
    nch_bq = np.ceil(cnts.max(axis=0) / 128).astype(np.int64)  # [NB*4]
    nch_bq_2d = nch_bq.reshape(NB, NBANK)
    n_b = nch_bq_2d.sum(axis=1)                  # chunks per block
    chunk_base_b = np.zeros(NB + 1, dtype=np.int64)
    chunk_base_b[1:] = np.cumsum(n_b)
    TCH = int(chunk_base_b[-1])                  # total chunks (per layer)

    # gather instruction layout: ordered (g, q); within: blocks asc
    S_gq = np.zeros((NG, NBANK), dtype=np.int64)     # slots per instruction
    seg_choff = np.zeros((NB, NBANK), dtype=np.int64)  # chunk offset of
    for g in range(NG):                                # block b in instr (g,q)
        for q in range(NBANK):
            off = 0
            for b in _group_blocks(g):
                seg_choff[b, q] = off
                off += nch_bq_2d[b, q]
            S_gq[g, q] = off * 128
    inst_slotbase = np.zeros((NG, NBANK + 1), dtype=np.int64)
    flat = S_gq.reshape(-1)
    starts = np.zeros(NG * NBANK, dtype=np.int64)
    starts[1:] = np.cumsum(flat)[:-1]
    TOTSLOT = int(flat.sum())                    # = 128 * TCH

    # per-edge static targets (same formulas for every core, data varies)
    # meta col of chunk (b, q, j): chunk_base_b[b] + qoff_bq + j
    qoff_bq = np.zeros((NB, NBANK), dtype=np.int64)
    qoff_bq[:, 1:] = np.cumsum(nch_bq_2d, axis=1)[:, :-1]
    # gather slot base of segment (b,q): starts[(g*NBANK)+q] + seg_choff*128
    gof = np.zeros((NB, NBANK), dtype=np.int64)
    for b in range(NB):
        g = b // G
        for q in range(NBANK):
            gof[b, q] = starts[g * NBANK + q] + seg_choff[b, q] * 128

    idx16 = np.zeros((NCORES, 128, TOTSLOT // 16), dtype=np.int16)
    dstL = np.full((NCORES, 128, TCH), -1.0, dtype=np.float32)
    for c in range(NCORES):
        lo, hi, dloc, key = percore[c]
        order2 = np.argsort(key, kind="stable")
        kk = key[order2]
        seg_starts = np.zeros(NB * NBANK, dtype=np.int64)
        seg_starts[1:] = np.cumsum(cnts[c])[:-1]
        pos = np.arange(hi - lo) - seg_starts[kk]
        b2, q2 = kk // NBANK, kk % NBANK
        j = pos >> 7
        p = pos & 127
        t = chunk_base_b[b2] + qoff_bq[b2, q2] + j
        dstL[c, p, t] = (dloc[order2] & 127).astype(np.float32)
        s_glob = gof[b2, q2] + pos
        inbank = (ss[lo:hi][order2] // NBANK).astype(np.int16)
        cc = s_glob >> 4
        pp = s_glob & 15
        for r in range(8):
            idx16[c, pp + 16 * r, cc] = inbank
    # per-core dinv arrays for the folded normalization: norm_e =
    # dinv[src]*dinv[dst]; dinv[src] is pre-scaled into the gather tables,
    # dinv[dst] (and the layer-1 bias + layer-2 src scale) in the tails.
    bf = mybir.dt.np(BF16)
    dinvC = np.zeros((NCORES, 128, NB), dtype=np.float32)
    dinvBT = np.zeros((NCORES, 128, NPAD), dtype=bf)
    dinv2BT = np.zeros((NCORES, 128, NPAD), dtype=bf)
    rdinvR = np.zeros((NCORES, 1, NPAD), dtype=bf)
    for c in range(NCORES):
        dv = np.zeros(NPAD, dtype=np.float32)
        dv[:NPC] = dinv[c * NPC:(c + 1) * NPC]
        dinvC[c] = dv.reshape(NB, 128).T
        dinvBT[c] = np.broadcast_to(dv.astype(bf), (128, NPAD))
        dinv2BT[c] = np.broadcast_to((dv * dv).astype(bf), (128, NPAD))
        rv = np.zeros(NPAD, dtype=np.float32)
        rv[:NPC] = np.sqrt(deg[c * NPC:(c + 1) * NPC])
        rdinvR[c] = rv.astype(bf)[None, :]
    return dict(TCH=TCH, TOTSLOT=TOTSLOT, nch_bq=nch_bq_2d, n_b=n_b,
                chunk_base_b=chunk_base_b, qoff_bq=qoff_bq,
                seg_choff=seg_choff, S_gq=S_gq, starts=starts,
                idx16=idx16, dstL=dstL.astype(bf), dinvC=dinvC,
                dinvBT=dinvBT, dinv2BT=dinv2BT, rdinvR=rdinvR)


def build_gcn(plan):
    TCH, TOTSLOT = plan["TCH"], plan["TOTSLOT"]
    nch_bq = plan["nch_bq"]
    n_b = plan["n_b"]
    chunk_base_b = plan["chunk_base_b"]
    qoff_bq = plan["qoff_bq"]
    seg_choff = plan["seg_choff"]
    S_gq = plan["S_gq"]
    starts = plan["starts"]

    nc = bacc.Bacc(None, num_swdge_queues=NBANK)
    xT = nc.dram_tensor("xT", [IN_DIM, NPAD], BF16, kind="ExternalInput")
    idx16 = nc.dram_tensor("idx16", [128, TOTSLOT // 16], I16,
                           kind="ExternalInput")
    dstL = nc.dram_tensor("dstL", [128, TCH], BF16, kind="ExternalInput")
    iota = nc.dram_tensor("iota", [128, 128], BF16, kind="ExternalInput")
    w1 = nc.dram_tensor("w1", [IN_DIM, HID], BF16, kind="ExternalInput")
    w2 = nc.dram_tensor("w2", [HID, HID], BF16, kind="ExternalInput")
    wm1 = nc.dram_tensor("wm1", [HID, MID], BF16, kind="ExternalInput")
    wm2 = nc.dram_tensor("wm2", [MID, OUT], BF16, kind="ExternalInput")
    b1r = nc.dram_tensor("b1r", [1, HID], BF16, kind="ExternalInput")
    b2c = nc.dram_tensor("b2c", [HID, 1], F32, kind="ExternalInput")
    bm1c = nc.dram_tensor("bm1c", [MID, 1], F32, kind="ExternalInput")
    bm2c = nc.dram_tensor("bm2c", [OUT, 1], F32, kind="ExternalInput")
    dinvC = nc.dram_tensor("dinvC", [128, NB], F32, kind="ExternalInput")
    dinvBT = nc.dram_tensor("dinvBT", [128, NPAD], BF16,
                            kind="ExternalInput")
    dinv2BT = nc.dram_tensor("dinv2BT", [128, NPAD], BF16,
                             kind="ExternalInput")
    rdinvR = nc.dram_tensor("rdinvR", [1, NPAD], BF16, kind="ExternalInput")
    outT = nc.dram_tensor("outT", [OUT, NPC], F32, kind="ExternalOutput")

    m0_loc = nc.dram_tensor("m0_loc", [NPC, HID], BF16)
    m0_full = nc.dram_tensor("m0_full", [N, HID], BF16, addr_space="Shared")
    m1_loc = nc.dram_tensor("m1_loc", [NPC, HID], BF16)
    m1_full = nc.dram_tensor("m1_full", [N, HID], BF16, addr_space="Shared")
    cc_sem = nc.alloc_semaphore(name="cc_sem")

    # -------- phase A: table rows T1 = dinv * (x @ W1) (node slice) -----
    with tile.TileContext(nc) as tc:
        with (
            tc.tile_pool(name="w1p", bufs=1) as w1p,
            tc.tile_pool(name="xtp", bufs=3) as xtp,
            tc.tile_pool(name="h0p", bufs=3) as h0p,
            tc.tile_pool(name="psA", bufs=3, space="PSUM") as psA,
        ):
            w1_sb = []
            for k0, ksz in KT:
                t = w1p.tile([ksz, HID], BF16, tag=f"w1_{k0}")
                nc.sync.dma_start(out=t[:], in_=w1[k0:k0 + ksz, :])
                w1_sb.append(t)
            dinvC_sb = w1p.tile([128, NB], F32, tag="dinvC")
            nc.sync.dma_start(out=dinvC_sb[:], in_=dinvC[:])
            n0 = 0
            while n0 < NPAD:
                W = min(512, NPAD - n0)
                xts = []
                for ti, (k0, ksz) in enumerate(KT):
                    t = xtp.tile([ksz, 512], BF16, tag=f"xt_{ti}")
                    nc.sync.dma_start(out=t[:, :W], in_=xT[k0:k0 + ksz, n0:n0 + W])
                    xts.append(t)
                for sub in range(0, W, 128):
                    ps = psA.tile([128, HID], F32, space="PSUM")
                    for ti in range(len(KT)):
                        nc.tensor.matmul(
                            out=ps[:],
                            lhsT=xts[ti][:, sub:sub + 128],
                            rhs=w1_sb[ti][:],
                            start=(ti == 0), stop=(ti == len(KT) - 1),
                        )
                    blk = (n0 + sub) // 128
                    h0 = h0p.tile([128, HID], BF16, tag="h0")
                    nc.scalar.activation(
                        out=h0[:], in_=ps[:],
                        func=mybir.ActivationFunctionType.Copy,
                        scale=dinvC_sb[:, blk:blk + 1])
                    rows = min(128, max(0, NPC - (n0 + sub)))
                    if rows > 0:
                        nc.sync.dma_start(
                            out=m0_loc[n0 + sub:n0 + sub + rows, :],
                            in_=h0[:rows, :])
                n0 += W

    nc.gpsimd.collective_compute(
        "AllGather", mybir.AluOpType.bypass,
        replica_groups=[list(range(NCORES))],
        ins=[m0_loc.ap().opt()], outs=[m0_full.ap().opt()],
    ).then_inc(cc_sem, 1)
    for eng in (nc.sync, nc.scalar, nc.vector, nc.tensor, nc.gpsimd):
        eng.wait_ge(cc_sem, 1)

    # ------------- aggregation sweep (shared by both layers) -------------
    def sweep(tc, pools, table, meta, tail, tail_closes_acc):
        gq, eP, psAcc = pools
        idx_sb, dstL_sb, iota_sb = meta
        gt = {}
        for g in range(NG):
            for q in range(NBANK):
                slots = int(S_gq[g, q])
                if slots == 0:
                    continue
                t = gq.tile([128, slots // 128, HID], BF16, tag=f"g{q}")
                base = int(starts[g * NBANK + q])
                nc.gpsimd.dma_gather(
                    out_ap=t[:, :, :],
                    in_ap=table.ap()[q::NBANK, :],
                    idxs_ap=idx_sb[:, base // 16:(base + slots) // 16],
                    num_idxs=slots,
                    num_idxs_reg=slots,
                    elem_size=HID,
                    elem_step=HID * NBANK,
                    single_packet=False,
                    queue_num=q,
                )
                gt[g, q] = t
            for b in _group_blocks(g):
                nb = int(n_b[b])
                t0 = int(chunk_base_b[b])
                eq = eP.tile([128, nb, 128], BF16, tag="eq")
                iota_bc = iota_sb[:, :].unsqueeze(1).broadcast_to(
                    [128, nb, 128])
                nc.vector.tensor_tensor(
                    out=eq[:, :, :], in0=iota_bc,
                    in1=dstL_sb[:, t0:t0 + nb].unsqueeze(2).broadcast_to(
                        [128, nb, 128]),
                    op=mybir.AluOpType.is_equal)
                acc = psAcc.tile([HID, 128], F32, space="PSUM")
                mm = 0
                for q in range(NBANK):
                    for j in range(int(nch_bq[b, q])):
                        nc.tensor.matmul(
                            out=acc[:],
                            lhsT=gt[b // G, q][:, int(seg_choff[b, q]) + j, :],
                            rhs=eq[:, int(qoff_bq[b, q]) + j, :],
                            start=(mm == 0),
                            stop=(not tail_closes_acc and mm == nb - 1),
                        )
                        mm += 1
                tail(b, acc)

    def load_meta(pool):
        idx_sb = pool.tile([128, TOTSLOT // 16], I16)
        nc.sync.dma_start(out=idx_sb[:], in_=idx16[:])
        dstL_sb = pool.tile([128, TCH], BF16)
        nc.sync.dma_start(out=dstL_sb[:], in_=dstL[:])
        iota_sb = pool.tile([128, 128], BF16)
        nc.sync.dma_start(out=iota_sb[:], in_=iota[:])
        return idx_sb, dstL_sb, iota_sb

    # ---------------- phase B: layer-1 aggregation + @W2 ----------------
    with tile.TileContext(nc) as tc:
        with (
            tc.tile_pool(name="metaB", bufs=1) as metaB,
            tc.tile_pool(name="gB", bufs=2) as gB,
            tc.tile_pool(name="eB", bufs=4) as eB,
            tc.tile_pool(name="psB", bufs=4, space="PSUM") as psB,
            tc.tile_pool(name="psB2", bufs=2, space="PSUM") as psB2,
            tc.tile_pool(name="hB", bufs=3) as hB,
            tc.tile_pool(name="wB", bufs=1) as wB,
        ):
            meta = load_meta(metaB)
            w2_sb = wB.tile([HID, HID], BF16)
            nc.sync.dma_start(out=w2_sb[:], in_=w2[:])
            b1r_sb = wB.tile([1, HID], BF16)
            nc.sync.dma_start(out=b1r_sb[:], in_=b1r[:])
            rdinv_sb = wB.tile([1, NPAD], BF16)
            nc.sync.dma_start(out=rdinv_sb[:], in_=rdinvR[:])
            dinv2_sb = wB.tile([128, NPAD], BF16)
            nc.sync.dma_start(out=dinv2_sb[:], in_=dinv2BT[:])

            def tail_B(b, acc):
                # acc += b1 (x) rdinv  (K=1 matmul closes the accumulation)
                nc.tensor.matmul(
                    out=acc[:], lhsT=b1r_sb[:],
                    rhs=rdinv_sb[:, b * 128:(b + 1) * 128],
                    start=False, stop=True)
                # v = dinv^2 * acc  -> T2 row block = (v^T @ W2)
                v = hB.tile([HID, 128], BF16, tag="v")
                nc.vector.scalar_tensor_tensor(
                    out=v[:], in0=acc[:], scalar=0.0,
                    in1=dinv2_sb[:, b * 128:(b + 1) * 128],
                    op0=mybir.AluOpType.add, op1=mybir.AluOpType.mult)
                m1ps = psB2.tile([128, HID], F32, space="PSUM")
                nc.tensor.matmul(out=m1ps[:], lhsT=v[:], rhs=w2_sb[:],
                                 start=True, stop=True)
                m1sb = hB.tile([128, HID], BF16, tag="m1sb")
                nc.scalar.activation(out=m1sb[:], in_=m1ps[:],
                                     func=mybir.ActivationFunctionType.Copy)
                rows = min(128, NPC - b * 128)
                nc.sync.dma_start(out=m1_loc[b * 128:b * 128 + rows, :],
                                  in_=m1sb[:rows, :])

            sweep(tc, (gB, eB, psB), m0_full, meta, tail_B, True)

    nc.gpsimd.collective_compute(
        "AllGather", mybir.AluOpType.bypass,
        replica_groups=[list(range(NCORES))],
        ins=[m1_loc.ap().opt()], outs=[m1_full.ap().opt()],
    ).then_inc(cc_sem, 1)
    for eng in (nc.sync, nc.scalar, nc.vector, nc.tensor, nc.gpsimd):
        eng.wait_ge(cc_sem, 2)

    # ---------------- phase C: layer-2 aggregation + MLP head ------------
    with tile.TileContext(nc) as tc:
        with (
            tc.tile_pool(name="metaC", bufs=1) as metaC,
            tc.tile_pool(name="gC", bufs=2) as gC,
            tc.tile_pool(name="eC", bufs=4) as eC,
            tc.tile_pool(name="psC", bufs=4, space="PSUM") as psC,
            tc.tile_pool(name="psC2", bufs=2, space="PSUM") as psC2,
            tc.tile_pool(name="psC3", bufs=2, space="PSUM") as psC3,
            tc.tile_pool(name="hC", bufs=3) as hC,
            tc.tile_pool(name="wC", bufs=1) as wC,
        ):
            meta = load_meta(metaC)
            wm1_sb = wC.tile([HID, MID], BF16)
            nc.sync.dma_start(out=wm1_sb[:], in_=wm1[:])
            wm2_sb = wC.tile([MID, OUT], BF16)
            nc.sync.dma_start(out=wm2_sb[:], in_=wm2[:])
            b2_sb = wC.tile([HID, 1], F32)
            nc.sync.dma_start(out=b2_sb[:], in_=b2c[:])
            bm1_sb = wC.tile([MID, 1], F32)
            nc.sync.dma_start(out=bm1_sb[:], in_=bm1c[:])
            bm2_sb = wC.tile([OUT, 1], F32)
            nc.sync.dma_start(out=bm2_sb[:], in_=bm2c[:])
            dinv_sb = wC.tile([128, NPAD], BF16)
            nc.sync.dma_start(out=dinv_sb[:], in_=dinvBT[:])

            def tail_C(b, acc):
                # w = dinv * acc ; r2T = Relu(w + b2)
                w = hC.tile([HID, 128], BF16, tag="w")
                nc.vector.scalar_tensor_tensor(
                    out=w[:], in0=acc[:], scalar=0.0,
                    in1=dinv_sb[:, b * 128:(b + 1) * 128],
                    op0=mybir.AluOpType.add, op1=mybir.AluOpType.mult)
                r2T = hC.tile([HID, 128], BF16, tag="r2T")
                nc.scalar.activation(
                    out=r2T[:], in_=w[:],
                    func=mybir.ActivationFunctionType.Relu, bias=b2_sb[:])
                y1 = psC2.tile([MID, 128], F32, space="PSUM")
                nc.tensor.matmul(out=y1[:], lhsT=wm1_sb[:], rhs=r2T[:],
                                 start=True, stop=True)
                r1T = hC.tile([MID, 128], BF16, tag="r1T")
                nc.scalar.activation(
                    out=r1T[:], in_=y1[:],
                    func=mybir.ActivationFunctionType.Relu, bias=bm1_sb[:])
                z = psC3.tile([OUT, 128], F32, space="PSUM")
                nc.tensor.matmul(out=z[:], lhsT=wm2_sb[:], rhs=r1T[:],
                                 start=True, stop=True)
                zb = hC.tile([OUT, 128], F32, tag="zb")
                nc.vector.tensor_scalar(
                    out=zb[:], in0=z[:], scalar1=bm2_sb[:], scalar2=None,
                    op0=mybir.AluOpType.add)
                rows = min(128, NPC - b * 128)
                nc.sync.dma_start(out=outT[:, b * 128:b * 128 + rows],
                                  in_=zb[:, :rows])

            sweep(tc, (gC, eC, psC), m1_full, meta, tail_C, False)

    nc.compile()
    return nc


def make_inmaps(plan, inputs):
    bf = mybir.dt.np(BF16)
    x = np.asarray(inputs["x"], dtype=np.float32)
    iota = np.broadcast_to(
        np.arange(128, dtype=np.float32), (128, 128)).astype(bf)
    w1 = np.asarray(inputs["w1"], np.float32).astype(bf)
    w2 = np.asarray(inputs["w2"], np.float32).astype(bf)
    wm1 = np.asarray(inputs["wm1"], np.float32).astype(bf)
    wm2 = np.asarray(inputs["wm2"], np.float32).astype(bf)
    b1r = np.asarray(inputs["b1"], np.float32).astype(bf)[None, :]
    b2c = np.asarray(inputs["b2"], np.float32)[:, None]
    bm1c = np.asarray(inputs["bm1"], np.float32)[:, None]
    bm2c = np.asarray(inputs["bm2"], np.float32)[:, None]
    in_maps = []
    for c in range(NCORES):
        xTc = np.zeros((IN_DIM, NPAD), dtype=bf)
        xTc[:, :NPC] = x[c * NPC:(c + 1) * NPC].T.astype(bf)
        in_maps.append({
            "xT": xTc, "idx16": plan["idx16"][c],
            "dstL": plan["dstL"][c], "iota": iota,
            "w1": w1, "w2": w2, "wm1": wm1, "wm2": wm2,
            "b1r": b1r, "b2c": b2c, "bm1c": bm1c, "bm2c": bm2c,
            "dinvC": plan["dinvC"][c], "dinvBT": plan["dinvBT"][c],
            "dinv2BT": plan["dinv2BT"][c], "rdinvR": plan["rdinvR"][c],
        })
    return in_maps


def build(inputs):
    plan = make_plan(np.asarray(inputs["edge_index"]))
    nc = build_gcn(plan)
    in_maps = make_inmaps(plan, inputs)
    return nc, in_maps


def assemble(results, cfg=None):
    return np.concatenate(
        [np.asarray(results[c]["outT"], dtype=np.float32).T
         for c in range(NCORES)], axis=0)


def kernel(**inputs):
    """Full-input entry point: returns [N, 4] float32."""
    nc, in_maps = build(inputs)
    from concourse.bass_utils import run_bass_kernel_spmd
    res = run_bass_kernel_spmd(nc, in_maps, core_ids=list(range(NCORES)))
    return assemble(res.results)


# revision 22
# speedup vs baseline: 3638.0825x; 1.0544x over previous
"""Trainium2 Bass kernel for a 2-layer GCN (GCNConv x2 + MLP head),
8-core SPMD via run_bass_kernel_spmd.

Entry point: kernel(**inputs) -> np.ndarray [100000, 4] float32.

v2 design (bf16 everywhere except accumulation):
- Nodes are partitioned across 8 cores (12500 each). Each core projects
  its slice (x @ W1 in bf16), slices are AllGathered (bf16) into a full
  [N, 128] bf16 gather table per core.
- Edges (+ self-loops) are partitioned by destination. Per core they are
  bucketed by (dst block of 128, src bank of 25000). Each (block, bank)
  segment is padded to a multiple of 128 "slots"; segments of G=8
  consecutive blocks in the same bank form one dma_gather instruction
  (~4-6K indices), spread over 4 SWDGE queues (queue = bank).
- Per destination block the gathered chunks are reduced with a
  selection-matrix matmul into a [HID, 128] PSUM accumulator:
  s[e, d] = (iota==dstloc_e) * norm_e is built in 2 batched DVE
  tensor_tensor passes per block (broadcast APs over all of the block's
  chunks), then one matmul per chunk accumulates feat x dst.
- Layer-1 tail fuses +b1 and @W2; layer-2 tail fuses the MLP head.
  Chunk counts per (block, bank) are maxed across cores so all 8 cores
  run an identical program; per-core pads carry weight 0.
"""

import numpy as np

import concourse.bacc as bacc
import concourse.mybir as mybir
import concourse.tile as tile

F32 = mybir.dt.float32
BF16 = mybir.dt.bfloat16
I16 = mybir.dt.int16

N, IN_DIM, HID, MID, OUT = 100000, 518, 128, 64, 4
NCORES = 8
NPC = N // NCORES            # 12500
NB = (NPC + 127) // 128      # 98
NPAD = NB * 128              # 12544
NBANK = 4
BR = N // NBANK              # 25000 rows per bank (int16-indexable)
G = 2                        # dst blocks per gather group
NG = (NB + G - 1) // G       # 13
KT = [(k, min(128, IN_DIM - k)) for k in range(0, IN_DIM, 128)]


def default_cfg():
    return dict(N=N, NCORES=NCORES, IN_DIM=IN_DIM, HID=HID, MID=MID, OUT=OUT)


def derive(cfg):
    cfg = dict(cfg)
    cfg["NPC"] = NPC
    cfg["NB"] = NB
    return cfg


def _group_blocks(g):
    return range(g * G, min((g + 1) * G, NB))


def make_plan(edge_index):
    """Host-side graph preprocessing.

    Returns dict with the static chunk structure (shared by all cores)
    and per-core idx16/dstL/nrmE arrays.
    """
    src = np.asarray(edge_index[0], dtype=np.int64)
    dst = np.asarray(edge_index[1], dtype=np.int64)
    loops = np.arange(N, dtype=np.int64)
    ss_all = np.concatenate([src, loops])
    dd_all = np.concatenate([dst, loops])

    deg = np.bincount(dd_all, minlength=N).astype(np.float64)
    dinv = 1.0 / np.sqrt(deg)
    nn_all = (dinv[ss_all] * dinv[dd_all]).astype(np.float32)

    order = np.argsort(dd_all, kind="stable")
    ss, dd, nn = ss_all[order], dd_all[order], nn_all[order]
    bounds = np.searchsorted(dd, np.arange(NCORES + 1) * NPC)

    percore = []
    cnts = np.zeros((NCORES, NB * NBANK), dtype=np.int64)
    for c in range(NCORES):
        lo, hi = bounds[c], bounds[c + 1]
        dloc = dd[lo:hi] - c * NPC
        blk = dloc >> 7
        bank = ss[lo:hi] % NBANK      # mod-interleaved banks (spreads
        key = blk * NBANK + bank      # self-loops evenly)
        cnts[c] = np.bincount(key, minlength=NB * NBANK)
        percore.append((lo, hi, dloc, key))

    # static chunk structure: per (block, bank) number of 128-slot chunks
    nch_bq = np.ceil(cnts.max(axis=0) / 128).astype(np.int64)  # [NB*4]
    nch_bq_2d = nch_bq.reshape(NB, NBANK)
    n_b = nch_bq_2d.sum(axis=1)                  # chunks per block
    chunk_base_b = np.zeros(NB + 1, dtype=np.int64)
    chunk_base_b[1:] = np.cumsum(n_b)
    TCH = int(chunk_base_b[-1])                  # total chunks (per layer)

    # gather instruction layout: ordered (g, q); within: blocks asc
    S_gq = np.zeros((NG, NBANK), dtype=np.int64)     # slots per instruction
    seg_choff = np.zeros((NB, NBANK), dtype=np.int64)  # chunk offset of
    for g in range(NG):                                # block b in instr (g,q)
        for q in range(NBANK):
            off = 0
            for b in _group_blocks(g):
                seg_choff[b, q] = off
                off += nch_bq_2d[b, q]
            S_gq[g, q] = off * 128
    inst_slotbase = np.zeros((NG, NBANK + 1), dtype=np.int64)
    flat = S_gq.reshape(-1)
    starts = np.zeros(NG * NBANK, dtype=np.int64)
    starts[1:] = np.cumsum(flat)[:-1]
    TOTSLOT = int(flat.sum())                    # = 128 * TCH

    # per-edge static targets (same formulas for every core, data varies)
    # meta col of chunk (b, q, j): chunk_base_b[b] + qoff_bq + j
    qoff_bq = np.zeros((NB, NBANK), dtype=np.int64)
    qoff_bq[:, 1:] = np.cumsum(nch_bq_2d, axis=1)[:, :-1]
    # gather slot base of segment (b,q): starts[(g*NBANK)+q] + seg_choff*128
    gof = np.zeros((NB, NBANK), dtype=np.int64)
    for b in range(NB):
        g = b // G
        for q in range(NBANK):
            gof[b, q] = starts[g * NBANK + q] + seg_choff[b, q] * 128

    idx16 = np.zeros((NCORES, 128, TOTSLOT // 16), dtype=np.int16)
    dstL = np.full((NCORES, 128, TCH), -1.0, dtype=np.float32)
    for c in range(NCORES):
        lo, hi, dloc, key = percore[c]
        order2 = np.argsort(key, kind="stable")
        kk = key[order2]
        seg_starts = np.zeros(NB * NBANK, dtype=np.int64)
        seg_starts[1:] = np.cumsum(cnts[c])[:-1]
        pos = np.arange(hi - lo) - seg_starts[kk]
        b2, q2 = kk // NBANK, kk % NBANK
        j = pos >> 7
        p = pos & 127
        t = chunk_base_b[b2] + qoff_bq[b2, q2] + j
        dstL[c, p, t] = (dloc[order2] & 127).astype(np.float32)
        s_glob = gof[b2, q2] + pos
        inbank = (ss[lo:hi][order2] // NBANK).astype(np.int16)
        cc = s_glob >> 4
        pp = s_glob & 15
        for r in range(8):
            idx16[c, pp + 16 * r, cc] = inbank
    # per-core dinv arrays for the folded normalization: norm_e =
    # dinv[src]*dinv[dst]; dinv[src] is pre-scaled into the gather tables,
    # dinv[dst] (and the layer-1 bias + layer-2 src scale) in the tails.
    bf = mybir.dt.np(BF16)
    dinvC = np.zeros((NCORES, 128, NB), dtype=np.float32)
    dinvBT = np.zeros((NCORES, 128, NPAD), dtype=bf)
    dinv2BT = np.zeros((NCORES, 128, NPAD), dtype=bf)
    rdinvR = np.zeros((NCORES, 1, NPAD), dtype=bf)
    for c in range(NCORES):
        dv = np.zeros(NPAD, dtype=np.float32)
        dv[:NPC] = dinv[c * NPC:(c + 1) * NPC]
        dinvC[c] = dv.reshape(NB, 128).T
        dinvBT[c] = np.broadcast_to(dv.astype(bf), (128, NPAD))
        dinv2BT[c] = np.broadcast_to((dv * dv).astype(bf), (128, NPAD))
        rv = np.zeros(NPAD, dtype=np.float32)
        rv[:NPC] = np.sqrt(deg[c * NPC:(c + 1) * NPC])
        rdinvR[c] = rv.astype(bf)[None, :]
    return dict(TCH=TCH, TOTSLOT=TOTSLOT, nch_bq=nch_bq_2d, n_b=n_b,
                chunk_base_b=chunk_base_b, qoff_bq=qoff_bq,
                seg_choff=seg_choff, S_gq=S_gq, starts=starts,
                idx16=idx16, dstL=dstL.astype(bf), dinvC=dinvC,
                dinvBT=dinvBT, dinv2BT=dinv2BT, rdinvR=rdinvR)


def build_gcn(plan):
    TCH, TOTSLOT = plan["TCH"], plan["TOTSLOT"]
    nch_bq = plan["nch_bq"]
    n_b = plan["n_b"]
    chunk_base_b = plan["chunk_base_b"]
    qoff_bq = plan["qoff_bq"]
    seg_choff = plan["seg_choff"]
    S_gq = plan["S_gq"]
    starts = plan["starts"]

    nc = bacc.Bacc(None, num_swdge_queues=NBANK)
    xT = nc.dram_tensor("xT", [IN_DIM, NPAD], BF16, kind="ExternalInput")
    idx16 = nc.dram_tensor("idx16", [128, TOTSLOT // 16], I16,
                           kind="ExternalInput")
    dstL = nc.dram_tensor("dstL", [128, TCH], BF16, kind="ExternalInput")
    iota = nc.dram_tensor("iota", [128, 128], BF16, kind="ExternalInput")
    w1 = nc.dram_tensor("w1", [IN_DIM, HID], BF16, kind="ExternalInput")
    w2 = nc.dram_tensor("w2", [HID, HID], BF16, kind="ExternalInput")
    wm1 = nc.dram_tensor("wm1", [HID, MID], BF16, kind="ExternalInput")
    wm2 = nc.dram_tensor("wm2", [MID, OUT], BF16, kind="ExternalInput")
    b1r = nc.dram_tensor("b1r", [1, HID], BF16, kind="ExternalInput")
    b2c = nc.dram_tensor("b2c", [HID, 1], F32, kind="ExternalInput")
    bm1c = nc.dram_tensor("bm1c", [MID, 1], F32, kind="ExternalInput")
    bm2c = nc.dram_tensor("bm2c", [OUT, 1], F32, kind="ExternalInput")
    dinvC = nc.dram_tensor("dinvC", [128, NB], F32, kind="ExternalInput")
    dinvBT = nc.dram_tensor("dinvBT", [128, NPAD], BF16,
                            kind="ExternalInput")
    dinv2BT = nc.dram_tensor("dinv2BT", [128, NPAD], BF16,
                             kind="ExternalInput")
    rdinvR = nc.dram_tensor("rdinvR", [1, NPAD], BF16, kind="ExternalInput")
    outT = nc.dram_tensor("outT", [OUT, NPC], F32, kind="ExternalOutput")

    m0_loc = nc.dram_tensor("m0_loc", [NPC, HID], BF16)
    m0_full = nc.dram_tensor("m0_full", [N, HID], BF16, addr_space="Shared")
    m1_loc = nc.dram_tensor("m1_loc", [NPC, HID], BF16)
    m1_full = nc.dram_tensor("m1_full", [N, HID], BF16, addr_space="Shared")
    cc_sem = nc.alloc_semaphore(name="cc_sem")

    # -------- phase A: table rows T1 = dinv * (x @ W1) (node slice) -----
    with tile.TileContext(nc) as tc:
        with (
            tc.tile_pool(name="w1p", bufs=1) as w1p,
            tc.tile_pool(name="xtp", bufs=3) as xtp,
            tc.tile_pool(name="h0p", bufs=3) as h0p,
            tc.tile_pool(name="psA", bufs=3, space="PSUM") as psA,
        ):
            w1_sb = []
            for k0, ksz in KT:
                t = w1p.tile([ksz, HID], BF16, tag=f"w1_{k0}")
                nc.sync.dma_start(out=t[:], in_=w1[k0:k0 + ksz, :])
                w1_sb.append(t)
            dinvC_sb = w1p.tile([128, NB], F32, tag="dinvC")
            nc.sync.dma_start(out=dinvC_sb[:], in_=dinvC[:])
            n0 = 0
            while n0 < NPAD:
                W = min(512, NPAD - n0)
                xts = []
                for ti, (k0, ksz) in enumerate(KT):
                    t = xtp.tile([ksz, 512], BF16, tag=f"xt_{ti}")
                    nc.sync.dma_start(out=t[:, :W], in_=xT[k0:k0 + ksz, n0:n0 + W])
                    xts.append(t)
                for sub in range(0, W, 128):
                    ps = psA.tile([128, HID], F32, space="PSUM")
                    for ti in range(len(KT)):
                        nc.tensor.matmul(
                            out=ps[:],
                            lhsT=xts[ti][:, sub:sub + 128],
                            rhs=w1_sb[ti][:],
                            start=(ti == 0), stop=(ti == len(KT) - 1),
                        )
                    blk = (n0 + sub) // 128
                    h0 = h0p.tile([128, HID], BF16, tag="h0")
                    nc.scalar.activation(
                        out=h0[:], in_=ps[:],
                        func=mybir.ActivationFunctionType.Copy,
                        scale=dinvC_sb[:, blk:blk + 1])
                    rows = min(128, max(0, NPC - (n0 + sub)))
                    if rows > 0:
                        nc.sync.dma_start(
                            out=m0_loc[n0 + sub:n0 + sub + rows, :],
                            in_=h0[:rows, :])
                n0 += W

    nc.gpsimd.collective_compute(
        "AllGather", mybir.AluOpType.bypass,
        replica_groups=[list(range(NCORES))],
        ins=[m0_loc.ap().opt()], outs=[m0_full.ap().opt()],
    ).then_inc(cc_sem, 1)
    for eng in (nc.sync, nc.scalar, nc.vector, nc.tensor, nc.gpsimd):
        eng.wait_ge(cc_sem, 1)

    # ------------- aggregation sweep (shared by both layers) -------------
    def sweep(tc, pools, table, meta, tail, tail_closes_acc):
        gq, eP, psAcc = pools
        idx_sb, dstL_sb, iota_sb = meta
        gt = {}
        for g in range(NG):
            for q in range(NBANK):
                slots = int(S_gq[g, q])
                if slots == 0:
                    continue
                t = gq.tile([128, slots // 128, HID], BF16, tag=f"g{q}")
                base = int(starts[g * NBANK + q])
                nc.gpsimd.dma_gather(
                    out_ap=t[:, :, :],
                    in_ap=table.ap()[q::NBANK, :],
                    idxs_ap=idx_sb[:, base // 16:(base + slots) // 16],
                    num_idxs=slots,
                    num_idxs_reg=slots,
                    elem_size=HID,
                    elem_step=HID * NBANK,
                    single_packet=False,
                    queue_num=q,
                )
                gt[g, q] = t
            for b in _group_blocks(g):
                nb = int(n_b[b])
                t0 = int(chunk_base_b[b])
                eq = eP.tile([128, nb, 128], BF16, tag="eq")
                iota_bc = iota_sb[:, :].unsqueeze(1).broadcast_to(
                    [128, nb, 128])
                nc.vector.tensor_tensor(
                    out=eq[:, :, :], in0=iota_bc,
                    in1=dstL_sb[:, t0:t0 + nb].unsqueeze(2).broadcast_to(
                        [128, nb, 128]),
                    op=mybir.AluOpType.is_equal)
                acc = psAcc.tile([HID, 128], F32, space="PSUM")
                mm = 0
                for q in range(NBANK):
                    for j in range(int(nch_bq[b, q])):
                        nc.tensor.matmul(
                            out=acc[:],
                            lhsT=gt[b // G, q][:, int(seg_choff[b, q]) + j, :],
                            rhs=eq[:, int(qoff_bq[b, q]) + j, :],
                            start=(mm == 0),
                            stop=(not tail_closes_acc and mm == nb - 1),
                        )
                        mm += 1
                tail(b, acc)

    def load_meta(pool):
        idx_sb = pool.tile([128, TOTSLOT // 16], I16)
        nc.sync.dma_start(out=idx_sb[:], in_=idx16[:])
        dstL_sb = pool.tile([128, TCH], BF16)
        nc.sync.dma_start(out=dstL_sb[:], in_=dstL[:])
        iota_sb = pool.tile([128, 128], BF16)
        nc.sync.dma_start(out=iota_sb[:], in_=iota[:])
        return idx_sb, dstL_sb, iota_sb

    # ---------------- phase B: layer-1 aggregation + @W2 ----------------
    with tile.TileContext(nc) as tc:
        with (
            tc.tile_pool(name="metaB", bufs=1) as metaB,
            tc.tile_pool(name="gB", bufs=4) as gB,
            tc.tile_pool(name="eB", bufs=4) as eB,
            tc.tile_pool(name="psB", bufs=4, space="PSUM") as psB,
            tc.tile_pool(name="psB2", bufs=2, space="PSUM") as psB2,
            tc.tile_pool(name="hB", bufs=3) as hB,
            tc.tile_pool(name="wB", bufs=1) as wB,
        ):
            meta = load_meta(metaB)
            w2_sb = wB.tile([HID, HID], BF16)
            nc.sync.dma_start(out=w2_sb[:], in_=w2[:])
            b1r_sb = wB.tile([1, HID], BF16)
            nc.sync.dma_start(out=b1r_sb[:], in_=b1r[:])
            rdinv_sb = wB.tile([1, NPAD], BF16)
            nc.sync.dma_start(out=rdinv_sb[:], in_=rdinvR[:])
            dinv2_sb = wB.tile([128, NPAD], BF16)
            nc.sync.dma_start(out=dinv2_sb[:], in_=dinv2BT[:])

            def tail_B(b, acc):
                # acc += b1 (x) rdinv  (K=1 matmul closes the accumulation)
                nc.tensor.matmul(
                    out=acc[:], lhsT=b1r_sb[:],
                    rhs=rdinv_sb[:, b * 128:(b + 1) * 128],
                    start=False, stop=True)
                # v = dinv^2 * acc  -> T2 row block = (v^T @ W2)
                v = hB.tile([HID, 128], BF16, tag="v")
                nc.vector.scalar_tensor_tensor(
                    out=v[:], in0=acc[:], scalar=0.0,
                    in1=dinv2_sb[:, b * 128:(b + 1) * 128],
                    op0=mybir.AluOpType.add, op1=mybir.AluOpType.mult)
                m1ps = psB2.tile([128, HID], F32, space="PSUM")
                nc.tensor.matmul(out=m1ps[:], lhsT=v[:], rhs=w2_sb[:],
                                 start=True, stop=True)
                m1sb = hB.tile([128, HID], BF16, tag="m1sb")
                nc.scalar.activation(out=m1sb[:], in_=m1ps[:],
                                     func=mybir.ActivationFunctionType.Copy)
                rows = min(128, NPC - b * 128)
                nc.sync.dma_start(out=m1_loc[b * 128:b * 128 + rows, :],
                                  in_=m1sb[:rows, :])

            sweep(tc, (gB, eB, psB), m0_full, meta, tail_B, True)

    nc.gpsimd.collective_compute(
        "AllGather", mybir.AluOpType.bypass,
        replica_groups=[list(range(NCORES))],
        ins=[m1_loc.ap().opt()], outs=[m1_full.ap().opt()],
    ).then_inc(cc_sem, 1)
    for eng in (nc.sync, nc.scalar, nc.vector, nc.tensor, nc.gpsimd):
        eng.wait_ge(cc_sem, 2)

    # ---------------- phase C: layer-2 aggregation + MLP head ------------
    with tile.TileContext(nc) as tc:
        with (
            tc.tile_pool(name="metaC", bufs=1) as metaC,
            tc.tile_pool(name="gC", bufs=4) as gC,
            tc.tile_pool(name="eC", bufs=4) as eC,
            tc.tile_pool(name="psC", bufs=4, space="PSUM") as psC,
            tc.tile_pool(name="psC2", bufs=2, space="PSUM") as psC2,
            tc.tile_pool(name="psC3", bufs=2, space="PSUM") as psC3,
            tc.tile_pool(name="hC", bufs=3) as hC,
            tc.tile_pool(name="wC", bufs=1) as wC,
        ):
            meta = load_meta(metaC)
            wm1_sb = wC.tile([HID, MID], BF16)
            nc.sync.dma_start(out=wm1_sb[:], in_=wm1[:])
            wm2_sb = wC.tile([MID, OUT], BF16)
            nc.sync.dma_start(out=wm2_sb[:], in_=wm2[:])
            b2_sb = wC.tile([HID, 1], F32)
            nc.sync.dma_start(out=b2_sb[:], in_=b2c[:])
            bm1_sb = wC.tile([MID, 1], F32)
            nc.sync.dma_start(out=bm1_sb[:], in_=bm1c[:])
            bm2_sb = wC.tile([OUT, 1], F32)
            nc.sync.dma_start(out=bm2_sb[:], in_=bm2c[:])
            dinv_sb = wC.tile([128, NPAD], BF16)
            nc.sync.dma_start(out=dinv_sb[:], in_=dinvBT[:])

            def tail_C(b, acc):
                # w = dinv * acc ; r2T = Relu(w + b2)
                w = hC.tile([HID, 128], BF16, tag="w")
                nc.vector.scalar_tensor_tensor(
                    out=w[:], in0=acc[:], scalar=0.0,
                    in1=dinv_sb[:, b * 128:(b + 1) * 128],
                    op0=mybir.AluOpType.add, op1=mybir.AluOpType.mult)
                r2T = hC.tile([HID, 128], BF16, tag="r2T")
                nc.scalar.activation(
                    out=r2T[:], in_=w[:],
                    func=mybir.ActivationFunctionType.Relu, bias=b2_sb[:])
                y1 = psC2.tile([MID, 128], F32, space="PSUM")
                nc.tensor.matmul(out=y1[:], lhsT=wm1_sb[:], rhs=r2T[:],
                                 start=True, stop=True)
                r1T = hC.tile([MID, 128], BF16, tag="r1T")
                nc.scalar.activation(
                    out=r1T[:], in_=y1[:],
                    func=mybir.ActivationFunctionType.Relu, bias=bm1_sb[:])
                z = psC3.tile([OUT, 128], F32, space="PSUM")
                nc.tensor.matmul(out=z[:], lhsT=wm2_sb[:], rhs=r1T[:],
                                 start=True, stop=True)
                zb = hC.tile([OUT, 128], F32, tag="zb")
                nc.vector.tensor_scalar(
                    out=zb[:], in0=z[:], scalar1=bm2_sb[:], scalar2=None,
                    op0=mybir.AluOpType.add)
                rows = min(128, NPC - b * 128)
                nc.sync.dma_start(out=outT[:, b * 128:b * 128 + rows],
                                  in_=zb[:, :rows])

            sweep(tc, (gC, eC, psC), m1_full, meta, tail_C, False)

    nc.compile()
    return nc


def make_inmaps(plan, inputs):
    bf = mybir.dt.np(BF16)
    x = np.asarray(inputs["x"], dtype=np.float32)
    iota = np.broadcast_to(
        np.arange(128, dtype=np.float32), (128, 128)).astype(bf)
    w1 = np.asarray(inputs["w1"], np.float32).astype(bf)
    w2 = np.asarray(inputs["w2"], np.float32).astype(bf)
    wm1 = np.asarray(inputs["wm1"], np.float32).astype(bf)
    wm2 = np.asarray(inputs["wm2"], np.float32).astype(bf)
    b1r = np.asarray(inputs["b1"], np.float32).astype(bf)[None, :]
    b2c = np.asarray(inputs["b2"], np.float32)[:, None]
    bm1c = np.asarray(inputs["bm1"], np.float32)[:, None]
    bm2c = np.asarray(inputs["bm2"], np.float32)[:, None]
    in_maps = []
    for c in range(NCORES):
        xTc = np.zeros((IN_DIM, NPAD), dtype=bf)
        xTc[:, :NPC] = x[c * NPC:(c + 1) * NPC].T.astype(bf)
        in_maps.append({
            "xT": xTc, "idx16": plan["idx16"][c],
            "dstL": plan["dstL"][c], "iota": iota,
            "w1": w1, "w2": w2, "wm1": wm1, "wm2": wm2,
            "b1r": b1r, "b2c": b2c, "bm1c": bm1c, "bm2c": bm2c,
            "dinvC": plan["dinvC"][c], "dinvBT": plan["dinvBT"][c],
            "dinv2BT": plan["dinv2BT"][c], "rdinvR": plan["rdinvR"][c],
        })
    return in_maps


def build(inputs):
    plan = make_plan(np.asarray(inputs["edge_index"]))
    nc = build_gcn(plan)
    in_maps = make_inmaps(plan, inputs)
    return nc, in_maps


def assemble(results, cfg=None):
    return np.concatenate(
        [np.asarray(results[c]["outT"], dtype=np.float32).T
         for c in range(NCORES)], axis=0)


def kernel(**inputs):
    """Full-input entry point: returns [N, 4] float32."""
    nc, in_maps = build(inputs)
    from concourse.bass_utils import run_bass_kernel_spmd
    res = run_bass_kernel_spmd(nc, in_maps, core_ids=list(range(NCORES)))
    return assemble(res.results)
